# revision 13
# baseline (speedup 1.0000x reference)
"""Trainium2 Bass kernel for CFContrastiveLoss.

Reference semantics (per sample of N=16 options, D=768 dims):
  - L2-normalize option embeddings
  - sim = pairwise cosine sims within the sample (16x16 gram)
  - max_neg[n] = max over negative-labeled columns of sim[n, :]
  - loss = mean over (positive rows of valid samples) of relu(max_neg + 0.3)

Key structural idea vs a plain block-diagonal gram: a row's label splits its
role.  A POSITIVE row only ever needs the max of its sims against its
sample's NEGATIVE rows; a negative row is only ever a max *candidate*.  So
instead of computing full 128x128 grams over batch-order row groups (each
sample's 16x16 block computed in full), the host packs ~16 whole samples per
"block": their positive rows become the 128-column STATIONARY side and their
negative rows the 128-column MOVING side of pos x neg rectangular matmuls.
Each row is shipped exactly once (it is either positive or negative), so HBM
traffic is unchanged (~12.6 MB/core fp8), but the TensorEngine does HALF the
matmuls: per core ~65 blocks x 3 DoubleRow matmuls instead of 128 groups x 3.
The kernel moves from PE-bound to DMA-bound.

Details:
  - Host normalizes embeddings, scales by 16 (power of two) and casts to
    fp8 e4m3.  Sims come out scaled by 256; host divides it back out.
  - Packing: greedy two-pointer over valid samples sorted by pos-neg count
    imbalance, closing a block when either side would exceed 128 columns.
    A sample's positive rows may SPLIT across two blocks (its negatives are
    replicated); negative sets always stay whole so every pos row sees all
    of its sample's negatives.  Invalid samples (no pos or no neg) are
    dropped entirely, which implements the reference validity mask exactly.
    Blocks are dealt round-robin to the 8 cores; all cores run the same
    program with BLOCKS = ceil(total/8) blocks (tail blocks are zero pads).
  - Per block b, chunk pair k: matmul(PSUM[:, b%4, :], pt[:, 2k:2k+2, :],
    nt[:, 2k:2k+2, :], DoubleRow) accumulates the 768-dim contraction in 3
    fp8 DoubleRow matmuls (fp32 PSUM).  start=True on a PSUM bank's first
    matmul marks the whole bank pending-zero; later blocks overwrite via
    has_written bits.
  - Cross-sample (pos,neg) pairs inside a block are killed by ONE sentinel
    matmul per PSUM bank (4 blocks, 512 columns) accumulated LAST into the
    bank: mask[p,c] = -S^2*1*1^T + S^2*sum_t a_t(p) b_t(c), with S=128
    (fp8-exact), t running over the bank's ~66 samples, a_t indicator over
    the owning block's pos partitions and b_t over that block's 512-column
    slice only (so cross-block terms vanish).  The +-2^14 contributions
    cancel exactly inside the PE fp32 adder for same-sample pairs and push
    cross-sample/padding sims to <= -2^14+256.  One bank-wide mask (vs one
    per block) matters: fragmenting the mask into 4 N=128 matmuls with
    their own LDWEIGHTS leaves the PE array idle often enough that the HAM
    clock gate never opens and the whole kernel runs at 1.2 GHz.
  - Per PSUM bank one VectorE reduce_max over [128, 4, 128] (innermost
    axis) -> [128, 4] row maxes, accumulated in one SBUF tile, stored with
    a single DMA at the end.  relu/margin/mean run on host.
  - DMA: per super-group of blocks, the pos stream rides one HWDGE ring and
    the neg stream the other; mask streams ride behind them (alternating).
    Tapered schedule (4,4,8,16,...) starts compute early and keeps the
    post-last-byte tail short.  No on-device memsets: mask operands are
    shipped at their exact rank, so nothing gates the first transfers.
"""

import os

import ml_dtypes
import numpy as np

import concourse.bass as bass
import concourse.mybir as mybir
from concourse import bacc, tile
from concourse.bass_utils import run_bass_kernel_spmd

FP8 = mybir.dt.float8e4
NP_FP8 = ml_dtypes.float8_e4m3
F32 = mybir.dt.float32
DOUBLE_ROW = mybir.MatmulPerfMode.DoubleRow

B, N, D = 8192, 16, 768
N_CORES = 8
ROWS = B * N                      # 131072
KCH = D // 128                    # 6 contraction chunks
GPB = 4                           # blocks per PSUM bank (512 f32 / 128)
SENT = np.float32(128.0)          # fp8-exact sentinel factor (128*128 = 2^14)
ESCALE = np.float32(16.0)         # fp8 embedding scale (power of two)
SIM_SCALE = ESCALE * ESCALE       # gram outputs are scaled by this
MARGIN = np.float32(0.3)
MAXS = 31                         # samples-per-block cap (bank rank <= 125)
SG_MAX = 16                       # largest super-group (SBUF tile size)

_CACHE: dict = {}

LAST_RESULT = None  # BassKernelResults of the most recent device run


def _schedule(blocks: int):
    """Tapered super-group schedule: multiples of 4 + <4 remainder at end."""
    main = (blocks // 4) * 4
    head = [4, 4, 8]
    tail = [8, 4, 4]
    if main < sum(head) + sum(tail) + 16:
        sched = [4] * (main // 4)
    else:
        mid_total = main - sum(head) - sum(tail)
        mid = [16] * (mid_total // 16)
        r = mid_total - 16 * len(mid)
        if r:
            mid.append(r)
        sched = head + mid + tail
    rem = blocks - sum(sched)
    if rem:
        sched.append(rem)
    assert sum(sched) == blocks and max(sched) <= SG_MAX
    return sched


def _build_program(blocks: int, rank: int) -> bass.Bass:
    nbanks = (blocks + GPB - 1) // GPB
    nc = bacc.Bacc(None)
    pt = nc.declare_dram_parameter("pt", [128, blocks * KCH, 128], FP8, isOutput=False)
    nt = nc.declare_dram_parameter("nt", [128, blocks * KCH, 128], FP8, isOutput=False)
    # Mask operands are rank-padded to the full 128 partitions: non-dense
    # (few-partition) transfers fragment into tiny packets that round-robin
    # against the embedding stream and take 10x longer than their size
    # suggests, and DMA semaphore reuse then serializes everything behind
    # them.  Dense 128-partition transfers ride the fast path.
    mlhs = nc.declare_dram_parameter("mlhs", [128, nbanks * 128], FP8, isOutput=False)
    mrhs = nc.declare_dram_parameter("mrhs", [128, blocks * 128], FP8, isOutput=False)
    out = nc.declare_dram_parameter("out", [128, blocks], F32, isOutput=True)

    with tile.TileContext(nc) as tc:
        with (
            tc.tile_pool(name="emb", bufs=1) as emb_pool,
            tc.tile_pool(name="const", bufs=1) as const_pool,
            tc.tile_pool(name="psum", bufs=8, space="PSUM") as psum_pool,
        ):
            wide = const_pool.tile([128, blocks], F32)
            # All mask operands are preloaded in two small transfers (~0.7 MB)
            # that complete during the fixed NEFF startup window, so no mask
            # matmul ever waits on the embedding stream's ring FIFO.
            ml_t = const_pool.tile([128, nbanks * 128], FP8)
            mr_t = const_pool.tile([128, blocks * 128], FP8)
            nc.sync.dma_start(ml_t[:, :], mlhs[:, :])
            nc.scalar.dma_start(mr_t[:, :], mrhs[:, :])
            # HAM warmup: a burst of dummy DoubleRow matmuls on a zeroed
            # scratch tile keeps the PE array busy through the startup
            # window so the clock gate opens (1.2 -> 2.4 GHz) before real
            # data arrives.  The scratch PSUM bank is never read.
            warm = const_pool.tile([128, 2, 128], FP8)
            nc.vector.memset(warm[:, :, :], 0.0)
            ps_w = psum_pool.tile([128, GPB, 128], F32, tag="ps")
            for i in range(28):
                nc.tensor.matmul(
                    ps_w[:, 0, :], warm[:, :, :], warm[:, :, :],
                    start=(i == 0), stop=(i == 27),
                    perf_mode=DOUBLE_ROW,
                )

            # Per-BANK DMA granularity (4 blocks = ~390 KB per ring per
            # bank): data arrives every ~1 us, so PE wait gaps stay far
            # below the ~3.4 us HAM re-throttle window and the post-stream
            # compute tail is a single bank.
            for b in range(nbanks):
                nb = min(GPB, blocks - b * GPB)
                pt_t = emb_pool.tile([128, GPB * KCH, 128], FP8, tag="pt", bufs=10)
                nt_t = emb_pool.tile([128, GPB * KCH, 128], FP8, tag="nt", bufs=10)
                nk = nb * KCH
                c0 = b * GPB * KCH
                ring = nc.sync if b % 2 == 0 else nc.scalar
                oring = nc.scalar if b % 2 == 0 else nc.sync
                ring.dma_start(pt_t[:, :nk, :], pt[:, c0:c0 + nk, :])
                oring.dma_start(nt_t[:, :nk, :], nt[:, c0:c0 + nk, :])
                ps = psum_pool.tile([128, GPB, 128], F32, tag="ps")  # one full PSUM bank
                for g in range(nb):
                    for k in range(KCH // 2):
                        kc = g * KCH + 2 * k
                        nc.tensor.matmul(
                            ps[:, g, :],
                            pt_t[:, kc:kc + 2, :],
                            nt_t[:, kc:kc + 2, :],
                            start=(g == 0 and k == 0), stop=False,
                            perf_mode=DOUBLE_ROW,
                        )
                # Bank-wide sentinel mask, accumulated LAST (a single wide
                # matmul keeps the PE array dense).
                mc = b * GPB * 128
                nc.tensor.matmul(
                    ps[:, :nb, :],
                    ml_t[:, b * 128:(b + 1) * 128],
                    mr_t[:, mc:mc + nb * 128],
                    start=False, stop=True,
                )
                nc.vector.reduce_max(
                    wide[:, b * GPB: b * GPB + nb],
                    ps[:, :nb, :], axis=mybir.AxisListType.X)
            nc.sync.dma_start(out[:, :], wide[:, :])
    nc.finalize()
    return nc


def _pack_blocks(labels2d: np.ndarray):
    """Greedy balanced packing of valid samples into (pos<=128, neg<=128) blocks.

    Returns a list of blocks; each block is a list of (sample_id, pos_row_ids).
    A sample's pos rows may split across two adjacent blocks (its negs are
    replicated); neg sets always stay whole.
    """
    pos_cnt = (labels2d == 1).sum(1)
    neg_cnt = (labels2d == 0).sum(1)
    valid = (pos_cnt > 0) & (neg_cnt > 0)
    idx = np.where(valid)[0]
    order = idx[np.argsort(pos_cnt[idx] - neg_cnt[idx], kind="stable")]
    pos_rows = {int(s): (s * N + np.where(labels2d[s] == 1)[0]).astype(np.int64)
                for s in order}

    blocks = []
    cur, psum, nsum = [], 0, 0
    i, j = 0, len(order) - 1
    carry = None                      # (sample, remaining pos row ids)
    while i <= j or carry is not None:
        if carry is not None:
            s, prem = carry
            ns = int(neg_cnt[s])
            if nsum + ns <= 128 and len(cur) < MAXS and psum < 128:
                take = min(len(prem), 128 - psum)
                cur.append((s, prem[:take])); psum += take; nsum += ns
                if take == len(prem):
                    carry = None
                else:
                    carry = (s, prem[take:])
                    blocks.append(cur); cur, psum, nsum = [], 0, 0
            else:
                blocks.append(cur); cur, psum, nsum = [], 0, 0
            continue
        cand, other = (j, i) if psum <= nsum else (i, j)
        placed = False
        for c in ([cand, other] if i != j else [i]):
            s = int(order[c]); ps, ns = int(pos_cnt[s]), int(neg_cnt[s])
            if nsum + ns <= 128 and len(cur) < MAXS:
                pr = pos_rows[s]
                if psum + ps <= 128:
                    cur.append((s, pr)); psum += ps; nsum += ns
                elif psum < 128:
                    take = 128 - psum
                    cur.append((s, pr[:take])); psum += take; nsum += ns
                    carry = (s, pr[take:])
                else:
                    continue
                if c == i:
                    i += 1
                else:
                    j -= 1
                placed = True
                if carry is not None:
                    blocks.append(cur); cur, psum, nsum = [], 0, 0
                break
        if not placed:
            blocks.append(cur); cur, psum, nsum = [], 0, 0
    if cur:
        blocks.append(cur)
    return blocks


def _build_core_maps(labels2d: np.ndarray):
    """Pack globally, deal blocks round-robin to cores, build index/slot maps."""
    blocks = _pack_blocks(labels2d)
    nb = len(blocks)
    n_blk = (nb + N_CORES - 1) // N_CORES
    cores = []
    for c in range(N_CORES):
        deal = blocks[c::N_CORES]
        P_IDX = np.full((n_blk, 128), -1, np.int64)
        N_IDX = np.full((n_blk, 128), -1, np.int64)
        PS_SLOT = np.full((n_blk, 128), -1, np.int16)
        NS_SLOT = np.full((n_blk, 128), -1, np.int16)
        NSAMP = np.zeros(n_blk, np.int32)
        for b, blk in enumerate(deal):
            p0 = 0
            n0 = 0
            for t, (s, prows) in enumerate(blk):
                npos = len(prows)
                P_IDX[b, p0:p0 + npos] = prows
                PS_SLOT[b, p0:p0 + npos] = t
                nrows = s * N + np.where(labels2d[s] == 0)[0]
                nneg = len(nrows)
                N_IDX[b, n0:n0 + nneg] = nrows
                NS_SLOT[b, n0:n0 + nneg] = t
                p0 += npos
                n0 += nneg
            NSAMP[b] = len(blk)
            assert p0 <= 128 and n0 <= 128
        cores.append(dict(P_IDX=P_IDX, N_IDX=N_IDX,
                          PS_SLOT=PS_SLOT, NS_SLOT=NS_SLOT, NSAMP=NSAMP))
    # global mask rank: 1 + max samples in any PSUM bank (4 consecutive blocks)
    rank = 1
    for m in cores:
        ns = m["NSAMP"]
        for k in range(0, len(ns), GPB):
            rank = max(rank, 1 + int(ns[k:k + GPB].sum()))
    assert rank <= 128
    return n_blk, rank, cores


def _build_masks(m: dict, n_blk: int, rank: int):
    """Bank-wide sentinel mask operands for one core (rank-padded to 128)."""
    assert rank <= 128
    nbanks = (n_blk + GPB - 1) // GPB
    ml = np.zeros((128, nbanks * 128), dtype=NP_FP8)
    mr = np.zeros((128, n_blk * 128), dtype=NP_FP8)
    ml[0, :] = SENT
    mr[0, :] = -SENT
    cols = np.arange(128)
    for b in range(n_blk):
        bank = b // GPB
        off = 1 + int(m["NSAMP"][bank * GPB:b].sum())
        ps = m["PS_SLOT"][b]
        sel = ps >= 0
        ml[off + ps[sel], bank * 128 + cols[sel]] = SENT
        ns = m["NS_SLOT"][b]
        sel = ns >= 0
        mr[off + ns[sel], b * 128 + cols[sel]] = SENT
    return ml, mr


def _to_layout(rows8: np.ndarray) -> np.ndarray:
    # [blocks, 128 rows, 768] -> [dim-in-chunk, block, chunk, row]
    nb = rows8.shape[0]
    return np.ascontiguousarray(
        rows8.reshape(nb, 128, KCH, 128).transpose(3, 0, 2, 1)
    ).reshape(128, nb * KCH, 128)


def kernel(embeddings: np.ndarray, labels: np.ndarray) -> np.ndarray:
    global LAST_RESULT
    assert embeddings.shape == (B, N, D)
    assert labels.shape == (B, N)

    X = np.asarray(embeddings, dtype=np.float32).reshape(ROWS, D)
    lab2 = np.asarray(labels).reshape(B, N)

    ss = np.square(X).sum(axis=1, dtype=np.float32)
    norms = np.sqrt(ss)
    Xn8 = (X * (ESCALE / np.maximum(norms, np.float32(1e-12)))[:, None]).astype(NP_FP8)
    Xz = np.vstack([Xn8, np.zeros((1, D), NP_FP8)])   # index -1 -> zero row

    n_blk, rank, cores = _build_core_maps(lab2)

    in_maps = []
    for m in cores:
        ml, mr = _build_masks(m, n_blk, rank)
        in_maps.append({
            "pt": _to_layout(Xz[m["P_IDX"]]),
            "nt": _to_layout(Xz[m["N_IDX"]]),
            "mlhs": ml,
            "mrhs": mr,
        })

    ck = (n_blk, rank)
    if ck not in _CACHE:
        _CACHE.clear()
        _CACHE[ck] = _build_program(n_blk, rank)
    nc = _CACHE[ck]

    trace = os.environ.get("BASS_KERNEL_TRACE", "0") == "1"
    res = run_bass_kernel_spmd(nc, in_maps, list(range(N_CORES)), trace=trace)
    LAST_RESULT = res

    loss_sum = 0.0
    count = 0
    for c, m in enumerate(cores):
        mx = np.asarray(res.results[c]["out"])          # [128, n_blk]
        trip = np.maximum(mx.T / SIM_SCALE + MARGIN, np.float32(0.0))
        w = m["P_IDX"] >= 0
        loss_sum += (trip * w).sum(dtype=np.float64)
        count += int(w.sum())
    loss = np.float32(np.float32(loss_sum) / np.float32(max(count, 1)))
    return np.asarray(loss, dtype=np.float32)


# revision 14
# speedup vs baseline: 1.0715x; 1.0715x over previous
"""Trainium2 Bass kernel for CFContrastiveLoss.

Reference semantics (per sample of N=16 options, D=768 dims):
  - L2-normalize option embeddings
  - sim = pairwise cosine sims within the sample (16x16 gram)
  - max_neg[n] = max over negative-labeled columns of sim[n, :]
  - loss = mean over (positive rows of valid samples) of relu(max_neg + 0.3)

Key structural idea vs a plain block-diagonal gram: a row's label splits its
role.  A POSITIVE row only ever needs the max of its sims against its
sample's NEGATIVE rows; a negative row is only ever a max *candidate*.  So
instead of computing full 128x128 grams over batch-order row groups (each
sample's 16x16 block computed in full), the host packs ~16 whole samples per
"block": their positive rows become the 128-column STATIONARY side and their
negative rows the 128-column MOVING side of pos x neg rectangular matmuls.
Each row is shipped exactly once (it is either positive or negative), so HBM
traffic is unchanged (~12.6 MB/core fp8), but the TensorEngine does HALF the
matmuls: per core ~65 blocks x 3 DoubleRow matmuls instead of 128 groups x 3.
The kernel moves from PE-bound to DMA-bound.

Details:
  - Host normalizes embeddings, scales by 16 (power of two) and casts to
    fp8 e4m3.  Sims come out scaled by 256; host divides it back out.
  - Packing: greedy two-pointer over valid samples sorted by pos-neg count
    imbalance, closing a block when either side would exceed 128 columns.
    A sample's positive rows may SPLIT across two blocks (its negatives are
    replicated); negative sets always stay whole so every pos row sees all
    of its sample's negatives.  Invalid samples (no pos or no neg) are
    dropped entirely, which implements the reference validity mask exactly.
    Blocks are dealt round-robin to the 8 cores; all cores run the same
    program with BLOCKS = ceil(total/8) blocks (tail blocks are zero pads).
  - Per block b, chunk pair k: matmul(PSUM[:, b%4, :], pt[:, 2k:2k+2, :],
    nt[:, 2k:2k+2, :], DoubleRow) accumulates the 768-dim contraction in 3
    fp8 DoubleRow matmuls (fp32 PSUM).  start=True on a PSUM bank's first
    matmul marks the whole bank pending-zero; later blocks overwrite via
    has_written bits.
  - Cross-sample (pos,neg) pairs inside a block are killed by ONE sentinel
    matmul per PSUM bank (4 blocks, 512 columns) accumulated LAST into the
    bank: mask[p,c] = -S^2*1*1^T + S^2*sum_t a_t(p) b_t(c), with S=128
    (fp8-exact), t running over the bank's ~66 samples, a_t indicator over
    the owning block's pos partitions and b_t over that block's 512-column
    slice only (so cross-block terms vanish).  The +-2^14 contributions
    cancel exactly inside the PE fp32 adder for same-sample pairs and push
    cross-sample/padding sims to <= -2^14+256.  One bank-wide mask (vs one
    per block) matters: fragmenting the mask into 4 N=128 matmuls with
    their own LDWEIGHTS leaves the PE array idle often enough that the HAM
    clock gate never opens and the whole kernel runs at 1.2 GHz.
  - Per PSUM bank one VectorE reduce_max over [128, 4, 128] (innermost
    axis) -> [128, 4] row maxes, accumulated in one SBUF tile, stored with
    a single DMA at the end.  relu/margin/mean run on host.
  - DMA: per super-group of blocks, the pos stream rides one HWDGE ring and
    the neg stream the other; mask streams ride behind them (alternating).
    Tapered schedule (4,4,8,16,...) starts compute early and keeps the
    post-last-byte tail short.  No on-device memsets: mask operands are
    shipped at their exact rank, so nothing gates the first transfers.
"""

import os

import ml_dtypes
import numpy as np

import concourse.bass as bass
import concourse.mybir as mybir
from concourse import bacc, tile
from concourse.bass_utils import run_bass_kernel_spmd

FP8 = mybir.dt.float8e4
NP_FP8 = ml_dtypes.float8_e4m3
F32 = mybir.dt.float32
DOUBLE_ROW = mybir.MatmulPerfMode.DoubleRow

B, N, D = 8192, 16, 768
N_CORES = 8
ROWS = B * N                      # 131072
KCH = D // 128                    # 6 contraction chunks
GPB = 4                           # blocks per PSUM bank (512 f32 / 128)
SENT = np.float32(128.0)          # fp8-exact sentinel factor (128*128 = 2^14)
ESCALE = np.float32(16.0)         # fp8 embedding scale (power of two)
SIM_SCALE = ESCALE * ESCALE       # gram outputs are scaled by this
MARGIN = np.float32(0.3)
MAXS = 31                         # samples-per-block cap (bank rank <= 125)
SG_MAX = 16                       # largest super-group (SBUF tile size)

_CACHE: dict = {}

LAST_RESULT = None  # BassKernelResults of the most recent device run


def _schedule(blocks: int):
    """Tapered super-group schedule: multiples of 4 + <4 remainder at end."""
    main = (blocks // 4) * 4
    head = [4, 4, 8]
    tail = [8, 4, 4]
    if main < sum(head) + sum(tail) + 16:
        sched = [4] * (main // 4)
    else:
        mid_total = main - sum(head) - sum(tail)
        mid = [16] * (mid_total // 16)
        r = mid_total - 16 * len(mid)
        if r:
            mid.append(r)
        sched = head + mid + tail
    rem = blocks - sum(sched)
    if rem:
        sched.append(rem)
    assert sum(sched) == blocks and max(sched) <= SG_MAX
    return sched


def _build_program(blocks: int, rank: int) -> bass.Bass:
    nbanks = (blocks + GPB - 1) // GPB
    nc = bacc.Bacc(None)
    pt = nc.declare_dram_parameter("pt", [128, blocks * KCH, 128], FP8, isOutput=False)
    nt = nc.declare_dram_parameter("nt", [128, blocks * KCH, 128], FP8, isOutput=False)
    # Mask operands are rank-padded to the full 128 partitions: non-dense
    # (few-partition) transfers fragment into tiny packets that round-robin
    # against the embedding stream and take 10x longer than their size
    # suggests, and DMA semaphore reuse then serializes everything behind
    # them.  Dense 128-partition transfers ride the fast path.
    mlhs = nc.declare_dram_parameter("mlhs", [128, nbanks * 128], FP8, isOutput=False)
    mrhs = nc.declare_dram_parameter("mrhs", [128, blocks * 128], FP8, isOutput=False)
    out = nc.declare_dram_parameter("out", [128, blocks], F32, isOutput=True)

    with tile.TileContext(nc) as tc:
        with (
            tc.tile_pool(name="emb", bufs=1) as emb_pool,
            tc.tile_pool(name="const", bufs=1) as const_pool,
            tc.tile_pool(name="psum", bufs=8, space="PSUM") as psum_pool,
        ):
            wide = const_pool.tile([128, blocks], F32)
            # All mask operands are preloaded in two small transfers (~0.7 MB)
            # that complete during the fixed NEFF startup window, so no mask
            # matmul ever waits on the embedding stream's ring FIFO.
            ml_t = const_pool.tile([128, nbanks * 128], FP8)
            mr_t = const_pool.tile([128, blocks * 128], FP8)
            nc.sync.dma_start(ml_t[:, :], mlhs[:, :])
            nc.scalar.dma_start(mr_t[:, :], mrhs[:, :])
            # HAM warmup: a burst of dummy DoubleRow matmuls on a zeroed
            # scratch tile keeps the PE array busy through the startup
            # window so the clock gate opens (1.2 -> 2.4 GHz) before real
            # data arrives.  The scratch PSUM bank is never read.
            warm = const_pool.tile([128, 2, 128], FP8)
            nc.vector.memset(warm[:, :, :], 0.0)
            ps_w = psum_pool.tile([128, GPB, 128], F32, tag="ps")
            for i in range(28):
                nc.tensor.matmul(
                    ps_w[:, 0, :], warm[:, :, :], warm[:, :, :],
                    start=(i == 0), stop=(i == 27),
                    perf_mode=DOUBLE_ROW,
                )

            # Super-groups of 8 blocks (2 PSUM banks, ~790 KB per ring):
            # large enough for line-rate DMA, small enough that the PE's
            # wait gaps stay near the ~3.4 us HAM re-throttle window and
            # the post-stream compute tail is short.
            sched = [4] + [8] * ((blocks - 9) // 8)
            sched += [blocks - sum(sched)] if blocks - sum(sched) <= 8 else [8, blocks - sum(sched) - 8]
            g0 = 0
            for sg, ng in enumerate(sched):
                pt_t = emb_pool.tile([128, 8 * KCH, 128], FP8, tag="pt", bufs=8)
                nt_t = emb_pool.tile([128, 8 * KCH, 128], FP8, tag="nt", bufs=8)
                nk = ng * KCH
                c0 = g0 * KCH
                ring = nc.sync if sg % 2 == 0 else nc.scalar
                oring = nc.scalar if sg % 2 == 0 else nc.sync
                ring.dma_start(pt_t[:, :nk, :], pt[:, c0:c0 + nk, :])
                oring.dma_start(nt_t[:, :nk, :], nt[:, c0:c0 + nk, :])
                for bk in range((ng + GPB - 1) // GPB):
                    nb = min(GPB, ng - bk * GPB)
                    b = (g0 + bk * GPB) // GPB
                    ps = psum_pool.tile([128, GPB, 128], F32, tag="ps")  # one full PSUM bank
                    for g in range(nb):
                        for k in range(KCH // 2):
                            kc = (bk * GPB + g) * KCH + 2 * k
                            nc.tensor.matmul(
                                ps[:, g, :],
                                pt_t[:, kc:kc + 2, :],
                                nt_t[:, kc:kc + 2, :],
                                start=(g == 0 and k == 0), stop=False,
                                perf_mode=DOUBLE_ROW,
                            )
                    # Bank-wide sentinel mask, accumulated LAST (a single
                    # wide matmul keeps the PE array dense).
                    mc = b * GPB * 128
                    nc.tensor.matmul(
                        ps[:, :nb, :],
                        ml_t[:, b * 128:(b + 1) * 128],
                        mr_t[:, mc:mc + nb * 128],
                        start=False, stop=True,
                    )
                    nc.vector.reduce_max(
                        wide[:, b * GPB: b * GPB + nb],
                        ps[:, :nb, :], axis=mybir.AxisListType.X)
                g0 += ng
            nc.sync.dma_start(out[:, :], wide[:, :])
    nc.finalize()
    return nc


def _pack_blocks(labels2d: np.ndarray):
    """Greedy balanced packing of valid samples into (pos<=128, neg<=128) blocks.

    Returns a list of blocks; each block is a list of (sample_id, pos_row_ids).
    A sample's pos rows may split across two adjacent blocks (its negs are
    replicated); neg sets always stay whole.
    """
    pos_cnt = (labels2d == 1).sum(1)
    neg_cnt = (labels2d == 0).sum(1)
    valid = (pos_cnt > 0) & (neg_cnt > 0)
    idx = np.where(valid)[0]
    order = idx[np.argsort(pos_cnt[idx] - neg_cnt[idx], kind="stable")]
    pos_rows = {int(s): (s * N + np.where(labels2d[s] == 1)[0]).astype(np.int64)
                for s in order}

    blocks = []
    cur, psum, nsum = [], 0, 0
    i, j = 0, len(order) - 1
    carry = None                      # (sample, remaining pos row ids)
    while i <= j or carry is not None:
        if carry is not None:
            s, prem = carry
            ns = int(neg_cnt[s])
            if nsum + ns <= 128 and len(cur) < MAXS and psum < 128:
                take = min(len(prem), 128 - psum)
                cur.append((s, prem[:take])); psum += take; nsum += ns
                if take == len(prem):
                    carry = None
                else:
                    carry = (s, prem[take:])
                    blocks.append(cur); cur, psum, nsum = [], 0, 0
            else:
                blocks.append(cur); cur, psum, nsum = [], 0, 0
            continue
        cand, other = (j, i) if psum <= nsum else (i, j)
        placed = False
        for c in ([cand, other] if i != j else [i]):
            s = int(order[c]); ps, ns = int(pos_cnt[s]), int(neg_cnt[s])
            if nsum + ns <= 128 and len(cur) < MAXS:
                pr = pos_rows[s]
                if psum + ps <= 128:
                    cur.append((s, pr)); psum += ps; nsum += ns
                elif psum < 128:
                    take = 128 - psum
                    cur.append((s, pr[:take])); psum += take; nsum += ns
                    carry = (s, pr[take:])
                else:
                    continue
                if c == i:
                    i += 1
                else:
                    j -= 1
                placed = True
                if carry is not None:
                    blocks.append(cur); cur, psum, nsum = [], 0, 0
                break
        if not placed:
            blocks.append(cur); cur, psum, nsum = [], 0, 0
    if cur:
        blocks.append(cur)
    return blocks


def _build_core_maps(labels2d: np.ndarray):
    """Pack globally, deal blocks round-robin to cores, build index/slot maps."""
    blocks = _pack_blocks(labels2d)
    nb = len(blocks)
    n_blk = (nb + N_CORES - 1) // N_CORES
    cores = []
    for c in range(N_CORES):
        deal = blocks[c::N_CORES]
        P_IDX = np.full((n_blk, 128), -1, np.int64)
        N_IDX = np.full((n_blk, 128), -1, np.int64)
        PS_SLOT = np.full((n_blk, 128), -1, np.int16)
        NS_SLOT = np.full((n_blk, 128), -1, np.int16)
        NSAMP = np.zeros(n_blk, np.int32)
        for b, blk in enumerate(deal):
            p0 = 0
            n0 = 0
            for t, (s, prows) in enumerate(blk):
                npos = len(prows)
                P_IDX[b, p0:p0 + npos] = prows
                PS_SLOT[b, p0:p0 + npos] = t
                nrows = s * N + np.where(labels2d[s] == 0)[0]
                nneg = len(nrows)
                N_IDX[b, n0:n0 + nneg] = nrows
                NS_SLOT[b, n0:n0 + nneg] = t
                p0 += npos
                n0 += nneg
            NSAMP[b] = len(blk)
            assert p0 <= 128 and n0 <= 128
        cores.append(dict(P_IDX=P_IDX, N_IDX=N_IDX,
                          PS_SLOT=PS_SLOT, NS_SLOT=NS_SLOT, NSAMP=NSAMP))
    # global mask rank: 1 + max samples in any PSUM bank (4 consecutive blocks)
    rank = 1
    for m in cores:
        ns = m["NSAMP"]
        for k in range(0, len(ns), GPB):
            rank = max(rank, 1 + int(ns[k:k + GPB].sum()))
    assert rank <= 128
    return n_blk, rank, cores


def _build_masks(m: dict, n_blk: int, rank: int):
    """Bank-wide sentinel mask operands for one core (rank-padded to 128)."""
    assert rank <= 128
    nbanks = (n_blk + GPB - 1) // GPB
    ml = np.zeros((128, nbanks * 128), dtype=NP_FP8)
    mr = np.zeros((128, n_blk * 128), dtype=NP_FP8)
    ml[0, :] = SENT
    mr[0, :] = -SENT
    cols = np.arange(128)
    for b in range(n_blk):
        bank = b // GPB
        off = 1 + int(m["NSAMP"][bank * GPB:b].sum())
        ps = m["PS_SLOT"][b]
        sel = ps >= 0
        ml[off + ps[sel], bank * 128 + cols[sel]] = SENT
        ns = m["NS_SLOT"][b]
        sel = ns >= 0
        mr[off + ns[sel], b * 128 + cols[sel]] = SENT
    return ml, mr


def _to_layout(rows8: np.ndarray) -> np.ndarray:
    # [blocks, 128 rows, 768] -> [dim-in-chunk, block, chunk, row]
    nb = rows8.shape[0]
    return np.ascontiguousarray(
        rows8.reshape(nb, 128, KCH, 128).transpose(3, 0, 2, 1)
    ).reshape(128, nb * KCH, 128)


def kernel(embeddings: np.ndarray, labels: np.ndarray) -> np.ndarray:
    global LAST_RESULT
    assert embeddings.shape == (B, N, D)
    assert labels.shape == (B, N)

    X = np.asarray(embeddings, dtype=np.float32).reshape(ROWS, D)
    lab2 = np.asarray(labels).reshape(B, N)

    ss = np.square(X).sum(axis=1, dtype=np.float32)
    norms = np.sqrt(ss)
    Xn8 = (X * (ESCALE / np.maximum(norms, np.float32(1e-12)))[:, None]).astype(NP_FP8)
    Xz = np.vstack([Xn8, np.zeros((1, D), NP_FP8)])   # index -1 -> zero row

    n_blk, rank, cores = _build_core_maps(lab2)

    in_maps = []
    for m in cores:
        ml, mr = _build_masks(m, n_blk, rank)
        in_maps.append({
            "pt": _to_layout(Xz[m["P_IDX"]]),
            "nt": _to_layout(Xz[m["N_IDX"]]),
            "mlhs": ml,
            "mrhs": mr,
        })

    ck = (n_blk, rank)
    if ck not in _CACHE:
        _CACHE.clear()
        _CACHE[ck] = _build_program(n_blk, rank)
    nc = _CACHE[ck]

    trace = os.environ.get("BASS_KERNEL_TRACE", "0") == "1"
    res = run_bass_kernel_spmd(nc, in_maps, list(range(N_CORES)), trace=trace)
    LAST_RESULT = res

    loss_sum = 0.0
    count = 0
    for c, m in enumerate(cores):
        mx = np.asarray(res.results[c]["out"])          # [128, n_blk]
        trip = np.maximum(mx.T / SIM_SCALE + MARGIN, np.float32(0.0))
        w = m["P_IDX"] >= 0
        loss_sum += (trip * w).sum(dtype=np.float64)
        count += int(w.sum())
    loss = np.float32(np.float32(loss_sum) / np.float32(max(count, 1)))
    return np.asarray(loss, dtype=np.float32)
